# revision 1
# baseline (speedup 1.0000x reference)
"""DOSLoss Trainium2 kernel — ragged-packed, class-on-partition layout.

Full inputs in, scalar loss out. The two heavy per-row contractions run on
device; everything O(B*K) runs on host in float64.

Key ideas vs the naive per-sample kernel:
  * Ragged packing: only the sum(lengths) valid (b,k) rows are uploaded and
    processed (~half of B*Kmax for uniform lengths), load-balanced so every
    core gets ceil(V/8) rows regardless of per-sample lengths.
  * Class-on-partition layout: cls row r is stored as [125 partitions, 8]
    (c = p*8 + j, 1000 = 125*8 exactly). The ACT engine then does pure
    elementwise exp (8 elems/row of engine time — its roofline, 0.833ns/elem
    dtype-blind) with no per-instruction accumulator reads; the otherwise
    idle PE contracts classes+partitions with 8 accumulating ones-matmuls
    into PSUM [1, R] per block.
  * d2 via h = (n - 2f)*n summed by PE: host uploads n rows ([128, R, 2],
    d = p*2 + j) and 2*deep_feats replicated per row, interleaved in ONE
    dram tensor (one DMA issue per block; the 565ns/issue SP sequencer
    otherwise delays the cls stream). DVE computes h in two bf16 2x-mode
    tensor ops; PE ones-matmuls fold sum(n^2) - 2*f.n into PSUM. Host adds
    ||f||^2 in fp64.
  * dtypes: cls in fp8 e4m3 (exp(x) feeds a 1000-term sum; rounding is
    ~0.04 absolute on the logit -> ~2e-3 on lse, irrelevant at the 2e-2
    gate), n/2f in bf16. DMA drops to ~1.5KB/row; ACT cost is dtype-blind.
  * PSUM evacuation (DMA cannot read PSUM): DVE copies lag one block behind
    compute; the closing blocks' copies run on ACT after its exp stream
    ends (engine streams are in-order, DVE otherwise serializes the tail).
  * Schedule shaping: cls DMAs issue 3 chunks ahead of the nf stream; ramp
    blocks 128/256 keep exp fed during pipeline fill; a small 96-row block
    plus a rows-on-partition accum_out tail (exp's accumulator emits
    per-row sums straight to SBUF, skipping fold+copy) minimize the
    post-ACT dependency chain into the final DMA.

Steady state is ACT-roofline-bound (~2.7us per 376-row block); fill ~3.2us
and final copy->DMA->sem->barrier chain ~3.9us bookend it.
"""

import os
import time

import numpy as np

B, KMAX, D, C = 64, 512, 256, 1000
N_CORES = 8
PC = 125  # class partitions: C = PC * 8
JC = 8
PD = 128  # d partitions: D = PD * 2
JD = 2
MAX_RBLK = 448  # matmul moving-dim cap is 512; keep a multiple of 8 below it
TAIL_ROWS = 128  # rows handled by the rows-on-partition accum-out tail path

_CACHE = {}
LAST_RESULTS = None  # BassKernelResults of the most recent device run


def _plan(v_max):
    """Block plan for v_max rows per core: tuple of block row counts.

    Small first block shortens pipeline fill (first exp starts after a small
    cls DMA); small last block shortens the post-ACT tail chain (fold +
    matmul + PSUM copies of the final block). Middle blocks bounded by the
    512-row matmul moving-dim cap.
    """
    if v_max <= 512:
        return (((-(-v_max // 8) * 8,),), 0)
    # Plan = exp chunks; each chunk is one cls DMA + one exp instruction
    # (185ns init amortized) split into <=MAX_RBLK fold blocks (matmul
    # moving-dim cap). Ramp-up chunks keep the exp stream fed during
    # pipeline fill; the small closing block shortens the fold->copy->DMA
    # tail; the last TAIL_ROWS rows use the rows-on-partition accum layout.
    rem = max(8, v_max - TAIL_ROWS - 384 - 96)
    ramp = ((128,), (256,))
    down = ((96,),)
    nmid = max(1, -(-rem // MAX_RBLK))
    mid = -(-rem // (nmid * 8)) * 8
    chunks = tuple((mid,) for _ in range(nmid))
    return (ramp + chunks + down, TAIL_ROWS)


def _build_nc(chunks, tail_rows):
    import concourse.bacc as bacc
    import concourse.mybir as mybir
    import concourse.tile as tile

    f32 = mybir.dt.float32
    bf16 = mybir.dt.bfloat16
    f8 = mybir.dt.float8e4
    blocks = [b for ch in chunks for b in ch]
    csizes = [sum(ch) for ch in chunks]
    r_main = sum(csizes)
    nblk = len(blocks)

    nc = bacc.Bacc("TRN2", target_bir_lowering=False, debug=False)

    cls_t = nc.dram_tensor("cls8", [PC, r_main, JC], f8, kind="ExternalInput")
    nf_t = nc.dram_tensor("nf", [PD, r_main, 2 * JD], bf16, kind="ExternalInput")
    out_t = nc.dram_tensor("out", [2, r_main], f32, kind="ExternalOutput")
    if tail_rows:
        ctl_t = nc.dram_tensor(
            "cls_tl", [tail_rows, C], f8, kind="ExternalInput"
        )
        nftl_t = nc.dram_tensor(
            "nf_tl", [tail_rows, 2 * D], bf16, kind="ExternalInput"
        )
        otl_t = nc.dram_tensor(
            "out_tl", [tail_rows, 2], f32, kind="ExternalOutput"
        )

    with tile.TileContext(nc) as tc:
        with (
            tc.tile_pool(name="cls_pool", bufs=4) as cls_pool,
            tc.tile_pool(name="exp_pool", bufs=3) as exp_pool,
            tc.tile_pool(name="n_pool", bufs=3) as n_pool,
            tc.tile_pool(name="scr_pool", bufs=3) as scr_pool,
            tc.tile_pool(name="const_pool", bufs=1) as const_pool,
            tc.tile_pool(name="stage_pool", bufs=1) as stage_pool,
            tc.tile_pool(name="psum_pool", bufs=3, space="PSUM") as psum_pool,
        ):
            ones_c = const_pool.tile([PC, 1], bf16)
            ones_d = const_pool.tile([PD, 1], bf16)
            nc.vector.memset(ones_c, 1.0)
            nc.vector.memset(ones_d, 1.0)

            stage = stage_pool.tile([1, 2 * r_main], f32)

            # cls DMAs run two chunks ahead of the nf stream so exp(b) never
            # waits on a fetch that queued behind nf(b-1) on the DMA engines
            cstarts = [sum(csizes[:i]) for i in range(len(csizes))]
            ctiles = {}

            def issue_cls(ci):
                c0 = cstarts[ci]
                c1 = c0 + csizes[ci]
                ct = cls_pool.tile([PC, csizes[ci], JC], f8, tag=f"cls{ci % 4}")
                nc.sync.dma_start(out=ct, in_=cls_t.ap()[:, c0:c1, :])
                ctiles[ci] = ct

            for ci in range(min(3, len(chunks))):
                issue_cls(ci)

            deferred = []  # (r0, r1, psum_e, psum_d) awaiting evacuation
            prefix_end = sum(blocks[:-2]) if nblk > 2 else 0
            ctl = nftl = None
            b_idx = 0
            r0 = 0
            for ci, ch in enumerate(chunks):
                if ci + 3 < len(chunks):
                    issue_cls(ci + 3)
                if tail_rows and ci == min(2, len(chunks) - 1):
                    # tail inputs load mid-ramp: any earlier and their HWDGE
                    # slots delay the ramp cls fetches
                    ctl = cls_pool.tile([tail_rows, C], f8)
                    nc.sync.dma_start(out=ctl, in_=ctl_t.ap())
                    nftl = n_pool.tile([tail_rows, 2 * D], bf16)
                    nc.sync.dma_start(out=nftl, in_=nftl_t.ap())

                csz = sum(ch)
                c1 = r0 + csz
                ctile = ctiles.pop(ci)
                nftile = n_pool.tile([PD, csz, 2 * JD], bf16, tag="nf")
                nc.sync.dma_start(out=nftile, in_=nf_t.ap()[:, r0:c1, :])

                # one exp instruction per chunk
                etile = exp_pool.tile([PC, csz, JC], bf16, tag="exp")
                nc.scalar.activation(
                    out=etile, in_=ctile,
                    func=mybir.ActivationFunctionType.Exp,
                )
                # d-path per chunk on DVE (2x bf16 mode)
                ntile = nftile[:, :, 0:JD]
                ftile = nftile[:, :, JD : 2 * JD]
                tdif = scr_pool.tile([PD, csz, JD], bf16, tag="td")
                nc.vector.tensor_sub(tdif, ntile, ftile)
                h = scr_pool.tile([PD, csz, JD], bf16, tag="h")
                nc.vector.tensor_mul(h, tdif, ntile)

                # fold blocks: PE contracts partitions into PSUM
                s0 = 0
                for r_blk in ch:
                    s1 = s0 + r_blk
                    psum_e = psum_pool.tile([1, r_blk], f32, tag="pe")
                    for j in range(JC):
                        nc.tensor.matmul(
                            psum_e, ones_c, etile[:, s0:s1, j],
                            start=(j == 0), stop=(j == JC - 1),
                        )
                    psum_d = psum_pool.tile([1, r_blk], f32, tag="pd")
                    for j in range(JD):
                        nc.tensor.matmul(
                            psum_d, ones_d, h[:, s0:s1, j],
                            start=(j == 0), stop=(j == JD - 1),
                        )
                    # evacuate PSUM (DMA cannot read it; engine streams are
                    # in-order). Copies lag one block behind on DVE.
                    deferred.append((r0 + s0, r0 + s1, psum_e, psum_d))
                    if len(deferred) > 1:
                        d0, d1, pe_t, pd_t = deferred.pop(0)
                        nc.vector.tensor_copy(stage[:, d0:d1], pe_t)
                        nc.vector.tensor_copy(
                            stage[:, r_main + d0 : r_main + d1], pd_t
                        )
                        if d1 == prefix_end:
                            # one 2-descriptor DMA ships both quantities'
                            # prefixes while the closing blocks are in flight
                            pre = stage[:, :].rearrange(
                                "p (q r) -> p q r", r=r_main
                            )[:, :, :d1]
                            nc.sync.dma_start(
                                out=out_t.ap()[:, :d1], in_=pre
                            )
                    s0 = s1
                    b_idx += 1
                r0 = c1

            if tail_rows:
                # tail path: rows on partitions; exp's accumulator gives the
                # per-row class sum directly (no PSUM round-trip), the d2
                # column is a short DVE chain, one tiny SP DMA ends the
                # kernel with no fold/copy on the critical tail.
                stage_tl = stage_pool.tile([tail_rows, 2], f32)
                etl = exp_pool.tile([tail_rows, C], bf16)
                nc.scalar.activation(
                    out=etl, in_=ctl,
                    func=mybir.ActivationFunctionType.Exp,
                    accum_out=stage_tl[:, 0:1],
                )
                ttd = scr_pool.tile([tail_rows, D], bf16, tag="ttd")
                nc.vector.tensor_sub(ttd, nftl[:, 0:D], nftl[:, D : 2 * D])
                tth = scr_pool.tile([tail_rows, D], bf16, tag="tth")
                nc.vector.tensor_mul(tth, ttd, nftl[:, 0:D])
                with nc.allow_low_precision("f32 accumulate"):
                    nc.vector.reduce_sum(
                        out=stage_tl[:, 1:2], in_=tth,
                        axis=mybir.AxisListType.X,
                    )
                nc.sync.dma_start(out=otl_t.ap(), in_=stage_tl)

            for d0, d1, pe_t, pd_t in deferred:
                nc.scalar.copy(stage[:, d0:d1], pe_t)
                nc.scalar.copy(stage[:, r_main + d0 : r_main + d1], pd_t)
            sfx = prefix_end
            suf = stage[:, :].rearrange("p (q r) -> p q r", r=r_main)[
                :, :, sfx:
            ]
            nc.sync.dma_start(out=out_t.ap()[:, sfx:], in_=suf)

    nc.compile()
    return nc


def _get_nc(key=None):
    if key is None:
        key = _CACHE.get("last_key")
        if key is None:
            key = _plan(-(-B * KMAX // N_CORES))
    if ("nc", key) not in _CACHE:
        _CACHE[("nc", key)] = _build_nc(*key)
    _CACHE["last_key"] = key
    return _CACHE[("nc", key)]


def _run_device(nc, in_maps):
    global LAST_RESULTS
    from concourse import bass_utils

    trace = bool(int(os.environ.get("DOS_TRACE", "0")))
    last_exc = None
    for _attempt in range(3):
        try:
            results = bass_utils.run_bass_kernel_spmd(
                nc, in_maps, core_ids=list(range(N_CORES)), trace=trace
            )
            break
        except Exception as e:
            last_exc = e
            time.sleep(5)
    else:
        raise last_exc
    LAST_RESULTS = results
    return list(results.results)


def kernel(deep_feats, n, w, cls_score, target, lengths):
    import ml_dtypes

    deep_feats = np.asarray(deep_feats, dtype=np.float32)
    n = np.asarray(n, dtype=np.float32)
    w = np.asarray(w, dtype=np.float32)
    cls_score = np.asarray(cls_score, dtype=np.float32)
    target = np.asarray(target).astype(np.int64)
    lengths = np.asarray(lengths).astype(np.int64)

    # packed stream of valid rows, ordered by (b, k)
    idx_b = np.repeat(np.arange(B), lengths)
    idx_k = np.concatenate([np.arange(l) for l in lengths])
    V = idx_b.shape[0]

    sizes = np.full(N_CORES, V // N_CORES, dtype=np.int64)
    sizes[: V % N_CORES] += 1
    starts = np.concatenate([[0], np.cumsum(sizes)])
    key = _plan(int(sizes.max()))
    chunks, tail_rows = key
    r_main = sum(sum(ch) for ch in chunks)
    r_pad = r_main + tail_rows

    f2 = 2.0 * deep_feats  # [B, D]
    in_maps = []
    for c in range(N_CORES):
        lo, hi = int(starts[c]), int(starts[c + 1])
        rb, rk = idx_b[lo:hi], idx_k[lo:hi]
        rc = hi - lo

        cls_rows = np.zeros((r_pad, C), dtype=np.float32)
        cls_rows[:rc] = cls_score[rb, rk]
        n_rows = np.zeros((r_pad, D), dtype=np.float32)
        n_rows[:rc] = n[rb, rk]
        f_rows = np.zeros((r_pad, D), dtype=np.float32)
        f_rows[:rc] = f2[rb]

        cls8 = np.ascontiguousarray(
            cls_rows[:r_main].reshape(r_main, PC, JC).transpose(1, 0, 2)
        ).astype(ml_dtypes.float8_e4m3fn)
        nf = np.empty((PD, r_main, 2 * JD), dtype=np.float32)
        nf[:, :, 0:JD] = (
            n_rows[:r_main].reshape(r_main, PD, JD).transpose(1, 0, 2)
        )
        nf[:, :, JD:] = (
            f_rows[:r_main].reshape(r_main, PD, JD).transpose(1, 0, 2)
        )
        nf = np.ascontiguousarray(nf).astype(ml_dtypes.bfloat16)
        im = {"cls8": cls8, "nf": nf}
        if tail_rows:
            im["cls_tl"] = cls_rows[r_main:].astype(ml_dtypes.float8_e4m3fn)
            nftl = np.concatenate(
                [n_rows[r_main:], f_rows[r_main:]], axis=1
            )
            im["nf_tl"] = nftl.astype(ml_dtypes.bfloat16)
        in_maps.append(im)

    outs = _run_device(_get_nc(key), in_maps)

    expsum = np.empty(V, dtype=np.float64)
    dpart = np.empty(V, dtype=np.float64)
    for c in range(N_CORES):
        lo, hi = int(starts[c]), int(starts[c + 1])
        o = np.asarray(outs[c]["out"], dtype=np.float64)  # [2, r_main]
        full = np.empty((2, r_pad), dtype=np.float64)
        full[:, :r_main] = o
        if tail_rows:
            otl = np.asarray(outs[c]["out_tl"], dtype=np.float64)
            full[:, r_main:] = otl.T
        expsum[lo:hi] = full[0, : hi - lo]
        dpart[lo:hi] = full[1, : hi - lo]

    # host tail in float64 over the packed stream
    ff2 = np.sum(deep_feats.astype(np.float64) ** 2, axis=1)  # [B]
    d2 = dpart + ff2[idx_b]
    dist = np.sqrt(np.maximum(d2, 0.0))
    wv = w[idx_b, idx_k].astype(np.float64)
    s = -wv * dist
    f_loss = float(np.sum(s))

    lse = np.log(np.maximum(expsum, 1e-300))
    cls_at = cls_score[idx_b, idx_k, target[idx_b]].astype(np.float64)
    ce = lse - cls_at

    # per-sample softmax of s over the ragged segments
    g_loss = 0.0
    for b in range(B):
        lo, hi = int(np.sum(lengths[:b])), int(np.sum(lengths[: b + 1]))
        sb = s[lo:hi]
        eb = np.exp(sb - sb.max())
        rho = eb / eb.sum()
        g_loss += float(np.sum(rho * ce[lo:hi]))

    return np.float32(f_loss + g_loss)

